# revision 31
# baseline (speedup 1.0000x reference)
"""Multi-head causal self-attention on 8 Trainium2 NeuronCores.

Problem: B=2, S=2048, E=1024, H=16 heads (D=64), causal mask, f32 I/O.

Sharding: (batch x head-group) -> 8 cores. Core c handles batch b=c//4 and
4 heads h0=4*(c%4).. (column-parallel Q/K/V projections, local attention,
row-parallel partial output projection). The 4 partial outputs per batch are
summed on the host (the "all-reduce" of row-parallel TP), where the output
bias bo and the folded V-bias term (bv @ Wo.T, exact because softmax rows
sum to 1) are also added. Partial outputs travel as bf16 (summed in f32 on
the host); the rel-error budget absorbs the rounding.

The attention phase is ScalarE-exp-throughput-bound (~75us of EXP), so the
program is software-pipelined so every other engine's work hides behind the
exp stream:
  - scores for a head pair land in one 2-bank PSUM tile [128, 2, 512];
    ONE ScalarE exp instruction covers both heads (ACT count matters).
  - The AV matmuls are emitted LAG=2 k-blocks behind the QK/exp front, so
    they never stall the PE queue waiting on an exp that hasn't run.
  - The causal-diagonal masking multiplies run on the otherwise-idle GpSimd
    engine (exact: exp(s)*0 == 0).
  - Each q-superblock's output projection + drain is spread into the next
    superblock's kb loop, keeping ScalarE fed during the PE-side work.

Device kernel layout (all matmuls bf16 with f32 PSUM accumulate):
  - Host pre-transposes activations/weights so the kernel never transposes:
      qhT/khT = Wq_h @ q[b].T  (projection emits [d, s] directly)
  - khT is per-head zero-padded: [128, 4, S]; head h occupies partitions
    (h%2)*64..+64 of slot h, the complementary half is zeros. qhT is packed
    as pair-slots [128, 2, S] (head 2m in partitions 0:64, head 2m+1 in
    64:128): the QK matmul contracts khT's zero half against the other
    head's q data, which contributes exactly zero. This keeps the Q-side
    PSUM drain a single full-width tensor_scalar_add.
  - attn^T feeds AV as the moving operand:
      ctx^T [d, q] = matmul(lhsT=V_aug [k, 128], rhs=attn^T)
    where V_aug cols 64:128 are ones, so rows 64:127 of the AV psum are the
    softmax row-sums pre-broadcast across 64 partitions (DVE cannot
    broadcast along partitions). Normalization is a DVE reciprocal+mul.
  - 1/sqrt(D) is folded into Wq/bq on the host.
  - Causal structure is exploited: only lower-triangular k-blocks are
    computed.
"""

import os
import sys

for _p in ("/opt/trn_rl_repo",):
    if _p not in sys.path and os.path.isdir(_p):
        sys.path.insert(0, _p)

import numpy as np
import ml_dtypes

import concourse.bacc as bacc
from concourse import mybir
from concourse.tile import TileContext
from concourse.bass_utils import run_bass_kernel_spmd

BF16 = ml_dtypes.bfloat16
P = 128
B, S, E, H, D = 2, 2048, 1024, 16, 64
HPC = 4            # heads per core
DC = HPC * D       # 256 output dims per core per projection
NCORES = 8
QSUP = 512         # q-superblock (matmul free dim)
NSUP = S // QSUP   # 4
NKB = S // P       # 16 k-blocks
LAG = 2            # AV runs LAG k-blocks behind the QK/exp front
SCALE = float(np.sqrt(D))

AF = mybir.ActivationFunctionType
f32 = mybir.dt.float32
bf16 = mybir.dt.bfloat16

_CACHE = {}
LAST = {}


def _install_axon_profile_shim():
    """Provide antenv.axon_hooks (absent in this image) so
    run_bass_kernel_spmd(trace=True) can NTFF-profile via libaxon_pjrt.so."""
    try:
        import antenv.axon_hooks  # noqa: F401
        return
    except ImportError:
        pass
    import contextlib
    import ctypes
    import types

    import antenv

    state = {"hook": None, "tried": False}

    def _build_hook():
        so_path = "/opt/axon/libaxon_pjrt.so"
        if not os.path.exists(so_path):
            return None
        lib = ctypes.CDLL(so_path)
        if not hasattr(lib, "axon_start_nrt_profile"):
            return None
        lib.axon_start_nrt_profile.argtypes = [
            ctypes.POINTER(ctypes.c_int64),
            ctypes.c_size_t,
        ]
        lib.axon_start_nrt_profile.restype = ctypes.c_int64
        lib.axon_stop_nrt_profile.argtypes = [ctypes.c_char_p]
        lib.axon_stop_nrt_profile.restype = ctypes.c_int64

        @contextlib.contextmanager
        def _hook(output_dir, device_ids):
            import jax

            jax.devices()
            if device_ids:
                ids = (ctypes.c_int64 * len(device_ids))(*device_ids)
                rc = lib.axon_start_nrt_profile(ids, len(device_ids))
            else:
                rc = lib.axon_start_nrt_profile(None, 0)
            if rc != 0:
                raise RuntimeError(f"axon_start_nrt_profile rc={rc}")
            try:
                yield
            finally:
                n = lib.axon_stop_nrt_profile(str(output_dir).encode())
                if n < 0:
                    raise RuntimeError(f"axon_stop_nrt_profile rc={n}")
                print(f"profile: {n} file(s) written to {output_dir}")

        return _hook

    mod = types.ModuleType("antenv.axon_hooks")

    def set_axon_ntff_profile_hook(h):
        state["hook"] = h
        state["tried"] = True

    def get_axon_ntff_profile_hook():
        if not state["tried"]:
            state["hook"] = _build_hook()
            state["tried"] = True
        return state["hook"]

    mod.set_axon_ntff_profile_hook = set_axon_ntff_profile_hook
    mod.get_axon_ntff_profile_hook = get_axon_ntff_profile_hook
    sys.modules["antenv.axon_hooks"] = mod
    antenv.axon_hooks = mod


_install_axon_profile_shim()


def _build_nc(causal: bool):
    nc = bacc.Bacc(None, target_bir_lowering=False)

    xqT = nc.dram_tensor("xqT", [E, S], bf16, kind="ExternalInput")
    xkT = nc.dram_tensor("xkT", [E, S], bf16, kind="ExternalInput")
    xvT = nc.dram_tensor("xvT", [E, S], bf16, kind="ExternalInput")
    wqT = nc.dram_tensor("wqT", [P, 8, DC], bf16, kind="ExternalInput")
    wkT = nc.dram_tensor("wkT", [P, 8, DC], bf16, kind="ExternalInput")
    wvT = nc.dram_tensor("wvT", [P, 8, DC], bf16, kind="ExternalInput")
    woT = nc.dram_tensor("woT", [P, 2, E], bf16, kind="ExternalInput")
    bqk = nc.dram_tensor("bqk", [P, 4], f32, kind="ExternalInput")
    cmask = nc.dram_tensor("cmask", [P, 2, P], bf16, kind="ExternalInput")
    out = nc.dram_tensor("out", [S, E], bf16, kind="ExternalOutput")

    with TileContext(nc) as tc:
        with (
            tc.tile_pool(name="consts", bufs=1) as consts,
            tc.tile_pool(name="xin", bufs=24) as xin,
            tc.tile_pool(name="acts", bufs=1) as acts,
            tc.tile_pool(name="attn", bufs=4) as attn,
            tc.tile_pool(name="norm", bufs=2) as norm,
            tc.tile_pool(name="osb", bufs=3) as osb,
            tc.tile_pool(name="ppool", bufs=2, space="PSUM") as ppool,
            tc.tile_pool(name="stp", bufs=2, space="PSUM") as stp,
            tc.tile_pool(name="cpool", bufs=2, space="PSUM") as cpool,
        ):
            # ---- HAM warm-up -----------------------------------------------
            # A dependency-free burst of matmuls on a zeroed scratch tile
            # warms the PE clock gate (~3.4us of activity needed) while the
            # first input DMAs stream (results are never read).
            warm = consts.tile([P, QSUP], bf16)
            nc.vector.memset(warm[:], 0.0)
            for wi in range(9):
                wp = ppool.tile([P, QSUP], f32, tag="ps", name=f"warm_{wi}")
                nc.tensor.matmul(wp, warm[:, 0:P], warm[:], start=True,
                                 stop=True)

            # ---- constants -------------------------------------------------
            wq_sb = consts.tile([P, 8, DC], bf16)
            wk_sb = consts.tile([P, 8, DC], bf16)
            wv_sb = consts.tile([P, 8, DC], bf16)
            wo_sb = consts.tile([P, 2, E], bf16)
            nc.sync.dma_start(wq_sb, wqT[:])
            bqk_sb = consts.tile([P, 4], f32)
            nc.sync.dma_start(bqk_sb[:], bqk[:])
            if causal:
                cm_sb = consts.tile([P, 2, P], bf16)
                nc.sync.dma_start(cm_sb[:], cmask[:])

            # ---- activations ----------------------------------------------
            # qhT pair-slots: [128, m, S]; head 2m in partitions 0:64, head
            # 2m+1 in 64:128 (matches the bias layout -> one full-width
            # tensor_scalar_add drains each Q PSUM chain).
            qhT = acts.tile([P, 2, S], bf16)
            # khT per-head zero-padded slots (the stationary side must be
            # zero-padded so the QK contraction kills the other head's data
            # in qhT). Pads are zeroed on the idle GpSimd engine; they are
            # only read ~50us later by the attention phase.
            khT = acts.tile([P, 4, S], bf16)
            for h in range(4):
                if h % 2 == 0:
                    nc.gpsimd.memset(khT[D:, h, :], 0.0)
                else:
                    nc.gpsimd.memset(khT[0:D, h, :], 0.0)
            # V natural layout + ones block: [:, sb, h, 0:64] = vh, 64:128 ones
            vha = acts.tile([P, NKB, HPC, 2 * D], bf16)
            ctxT = acts.tile([P, 2, S], bf16)
            nc.vector.memset(vha[:, :, :, D:], 1.0)

            # ---- Q/K/V projections ----------------------------------------
            def load_x(xT, t):
                xr = xT.rearrange("(ko p) s -> ko p s", p=P)
                tiles = []
                for ko in range(8):
                    tile = xin.tile([P, S], bf16, tag="xin",
                                    name=f"x_{t}_{ko}")
                    nc.sync.dma_start(tile, xr[ko])
                    tiles.append(tile)
                return tiles

            xt_q = load_x(xqT, "q")
            xt_k = load_x(xkT, "k")
            nc.sync.dma_start(wk_sb, wkT[:])
            nc.sync.dma_start(wv_sb, wvT[:])
            nc.sync.dma_start(wo_sb, woT[:])
            xt_v = load_x(xvT, "v")

            # ko-outer multi-chain projection pass: `chains[i]` accumulate
            # over ko as the x tiles land from DMA (tile ko is read right
            # when it arrives - the first pass is DMA-paced, not serialized
            # after it). Uses ps+cps PSUM slots; pre-attention only.
            def proj_pass(w_sb, xt, specs, drain):
                # specs: list of (name, x column slice); drain(i, chain)
                chains = []
                for i, (nm, _) in enumerate(specs):
                    pool, tag = ((ppool, "ps"), (cpool, "cps"))[i % 2]
                    chains.append(pool.tile([P, QSUP], f32, tag=tag,
                                            name=nm))
                for ko in range(8):
                    for i, (_, csl) in enumerate(specs):
                        nc.tensor.matmul(
                            chains[i],
                            w_sb[0][:, ko, w_sb[1] * P:(w_sb[1] + 1) * P]
                            if w_sb[1] is not None else w_sb[0][:, ko, :],
                            xt[ko][:, csl],
                            start=(ko == 0), stop=(ko == 7),
                        )
                for i in range(len(specs)):
                    drain(i, chains[i])

            def q_drain(m):
                def dr(i, ch):
                    nsl = slice(i * QSUP, (i + 1) * QSUP)
                    nc.vector.tensor_scalar_add(
                        qhT[:, m, nsl], ch, bqk_sb[:, m:m + 1],
                    )
                return dr

            def k_drain(m, base):
                def dr(i, ch):
                    ns = base + i
                    nsl = slice(ns * QSUP, (ns + 1) * QSUP)
                    bsl = bqk_sb[:, 2 + m:3 + m]
                    nc.vector.tensor_scalar_add(
                        khT[0:D, 2 * m, nsl], ch[0:D], bsl[0:D],
                    )
                    nc.vector.tensor_scalar_add(
                        khT[D:, 2 * m + 1, nsl], ch[D:], bsl[D:],
                    )
                return dr

            def kproj_chunk(ns, m):
                # single-chain filler form (x tiles already resident)
                def go():
                    ch = ppool.tile([P, QSUP], f32, tag="ps",
                                    name=f"k_{ns}_{m}")
                    for ko in range(8):
                        nc.tensor.matmul(
                            ch,
                            wk_sb[:, ko, m * P:(m + 1) * P],
                            xt_k[ko][:, ns * QSUP:(ns + 1) * QSUP],
                            start=(ko == 0), stop=(ko == 7),
                        )
                    k_drain(m, ns)(0, ch)
                return go

            def vproj_chunk(sb, pre=False):
                def go():
                    nm = f"vps_{sb}"
                    if pre:
                        pool, tag = ((ppool, "ps"), (cpool, "cps"))[sb % 2]
                    else:
                        pool, tag = ppool, "ps"
                    ps = pool.tile([P, DC], f32, tag=tag, name=nm)
                    for ko in range(8):
                        nc.tensor.matmul(
                            ps,
                            xt_v[ko][:, sb * P:(sb + 1) * P],
                            wv_sb[:, ko, :],
                            start=(ko == 0), stop=(ko == 7),
                        )
                    nc.vector.tensor_copy(
                        vha[:, sb, :, 0:D],
                        ps.rearrange("p (h d) -> p h d", h=HPC),
                    )
                return go

            def outproj_chunk(sb, split_drain=False, alt_pool=False):
                def go():
                    if alt_pool:
                        # tail-only: st2 banks are free once the last exp
                        # ran; alternating pools lets consecutive chunks'
                        # matmuls run back-to-back instead of waiting on
                        # the previous chunk's drain
                        ps2 = stp.tile([P, 2, QSUP], f32, tag="st2",
                                       name=f"pso2_{sb}")
                        pso = [ps2[:, 0, :], ps2[:, 1, :]]
                    else:
                        pso = [
                            ppool.tile([P, QSUP], f32, tag="ps",
                                       name=f"pso_{sb}_{n2}")
                            for n2 in range(2)
                        ]
                    for km in range(2):
                        for n2 in range(2):
                            nc.tensor.matmul(
                                pso[n2],
                                ctxT[:, km, sb * P:(sb + 1) * P],
                                wo_sb[:, km, n2 * QSUP:(n2 + 1) * QSUP],
                                start=(km == 0), stop=(km == 1),
                            )
                    # drain both halves into one [128, 1024] tile and DMA
                    # full DRAM rows: 2KB descriptors, half the descriptor
                    # count of two 512-wide stores
                    ot = osb.tile([P, 2, QSUP], bf16, tag="ot",
                                  name=f"ot_{sb}")
                    nc.vector.tensor_copy(out=ot[:, 0, :], in_=pso[0])
                    if split_drain:
                        # ScalarE is idle in the tail; halve the
                        # drain-copy serialization
                        nc.scalar.activation(ot[:, 1, :], pso[1], AF.Copy)
                    else:
                        nc.vector.tensor_copy(out=ot[:, 1, :], in_=pso[1])
                    nc.sync.dma_start(
                        out[sb * P:(sb + 1) * P, :],
                        ot.rearrange("p a q -> p (a q)"),
                    )
                return go

            # Up-front (DMA-paced, ko-outer multi-chain): full Q, K ns=0..1.
            # V interleaves with qs=0's QK/exp below; K ns=2..3, V sb=8..15
            # and every output projection are drip-fed into the attention
            # kb loops: pure-PE work that keeps the array busy while the
            # ScalarE exp stream (the attention bottleneck) drains.
            for m in range(2):
                proj_pass((wq_sb, m), xt_q,
                          [(f"q_{ns}_{m}",
                            slice(ns * QSUP, (ns + 1) * QSUP))
                           for ns in range(4)],
                          q_drain(m))
            for m in range(2):
                proj_pass((wk_sb, m), xt_k,
                          [(f"k_{ns}_{m}",
                            slice(ns * QSUP, (ns + 1) * QSUP))
                           for ns in range(2)],
                          k_drain(m, 0))

            def qlo_of(kb, qs):
                r = kb - 4 * qs
                return r * P if (causal and r >= 0) else 0

            def emit_qk(qs, m, kb):
                qlo = qlo_of(kb, qs)
                qsl = slice(qs * QSUP + qlo, (qs + 1) * QSUP)
                st = stp.tile([P, 2, QSUP], f32, tag="st2",
                              name=f"st_{m}_{qs}_{kb}")
                for h2 in range(2):
                    nc.tensor.matmul(
                        st[:, h2, qlo:],
                        khT[:, 2 * m + h2, kb * P:(kb + 1) * P],
                        qhT[:, m, qsl],
                        start=True, stop=True,
                    )
                at = attn.tile([P, 2, QSUP], bf16, tag="at",
                               name=f"at_{m}_{qs}_{kb}")
                nc.scalar.activation(at[:, :, qlo:], st[:, :, qlo:],
                                     AF.Exp)
                if causal and kb - 4 * qs >= 0:
                    nc.gpsimd.tensor_mul(
                        at[:, :, qlo:qlo + P], at[:, :, qlo:qlo + P],
                        cm_sb,
                    )
                return at

            def emit_av(qs, m, kb, cps, at, nkb):
                qlo = qlo_of(kb, qs)
                for h2 in range(2):
                    nc.tensor.matmul(
                        cps[h2][:, qlo:],
                        vha[:, kb, 2 * m + h2, :],
                        at[:, h2, qlo:],
                        start=(kb == 0), stop=(kb == nkb - 1),
                    )

            def mk_cps(qs, m):
                return [
                    cpool.tile([P, QSUP], f32, tag="cps",
                               name=f"cps_{m}_{qs}_{h2}")
                    for h2 in range(2)
                ]

            def emit_norm(qs, m, cps):
                # drain ctx + row-sums to SBUF first (4 half-width copies
                # free the cps accumulators for the next head pair ASAP),
                # then one reciprocal + one full-width mul into ctxT.
                cu = norm.tile([P, QSUP], bf16, tag="cu")
                su = norm.tile([P, QSUP], f32, tag="sums")
                nc.vector.tensor_copy(out=cu[0:D, :], in_=cps[0][0:D, :])
                nc.vector.tensor_copy(out=cu[D:, :], in_=cps[1][0:D, :])
                nc.vector.tensor_copy(out=su[0:D, :], in_=cps[0][D:, :])
                nc.vector.tensor_copy(out=su[D:, :], in_=cps[1][D:, :])
                rec = norm.tile([P, QSUP], f32, tag="rec")
                nc.vector.reciprocal_approx_fast(out=rec, in_=su)
                qsl = slice(qs * QSUP, (qs + 1) * QSUP)
                nc.vector.tensor_mul(ctxT[:, m, qsl], cu, rec)

            # fillers[(qs, m)] -> list of chunks to drip into that kb loop
            fillers = {}
            if causal:
                # K ns=2 fills the PE while the first V chunks wait on the
                # xv DMA (V is DMA-gated there regardless)
                kproj_chunk(2, 0)()
                kproj_chunk(2, 1)()
                for sb in range(8):
                    vproj_chunk(sb, pre=True)()
                fillers[(0, 0)] = [vproj_chunk(8), vproj_chunk(9),
                                   vproj_chunk(10)]
                fillers[(0, 1)] = [vproj_chunk(11), vproj_chunk(12)]
                fillers[(1, 0)] = ([outproj_chunk(sb) for sb in range(4)]
                                   + [kproj_chunk(3, 0), kproj_chunk(3, 1)])
                fillers[(1, 1)] = [vproj_chunk(13), vproj_chunk(14),
                                   vproj_chunk(15)]
                fillers[(2, 0)] = [outproj_chunk(sb) for sb in range(4, 8)]
                fillers[(3, 0)] = [outproj_chunk(sb) for sb in range(8, 12)]
                tail = [outproj_chunk(sb, split_drain=True)
                        for sb in range(12, 16)]
                qs_start = 0
            else:
                for m in range(2):
                    proj_pass((wk_sb, m), xt_k,
                              [(f"k_{ns}_{m}",
                                slice(ns * QSUP, (ns + 1) * QSUP))
                               for ns in range(2, 4)],
                              k_drain(m, 2))
                for sb in range(NKB):
                    vproj_chunk(sb, pre=True)()
                fillers[(1, 0)] = [outproj_chunk(sb) for sb in range(4)]
                fillers[(2, 0)] = [outproj_chunk(sb) for sb in range(4, 8)]
                fillers[(3, 0)] = [outproj_chunk(sb) for sb in range(8, 12)]
                tail = [outproj_chunk(sb, split_drain=True)
                        for sb in range(12, 16)]
                qs_start = 0

            # ---- attention -------------------------------------------------
            for qs in range(qs_start, NSUP):
                for m in range(2):        # head pair (local heads 2m, 2m+1)
                    nkb = 4 * qs + 4 if causal else NKB
                    drip = list(fillers.get((qs, m), []))
                    cps = mk_cps(qs, m)
                    ats = {}
                    for kb in range(nkb + LAG):
                        if kb < nkb:
                            ats[kb] = emit_qk(qs, m, kb)
                        if drip and kb >= 1:
                            drip.pop(0)()
                        j = kb - LAG
                        if 0 <= j < nkb:
                            emit_av(qs, m, j, cps, ats.pop(j), nkb)
                    for chunk in drip:   # leftovers (short kb loop)
                        chunk()
                    emit_norm(qs, m, cps)

            # Keep the PE clock warm across the final norm's DVE chain: a
            # >3.4us PE idle here re-throttles HAM to 1.2 GHz and the tail
            # output projections were measured running at cold clock.
            for wi in range(12):
                wp = ppool.tile([P, QSUP], f32, tag="ps",
                                name=f"twarm_{wi}")
                nc.tensor.matmul(wp, warm[:, 0:P], warm[:], start=True,
                                 stop=True)

            for chunk in tail:
                chunk()

    nc.finalize()
    return nc


def _get_nc(causal: bool):
    key = ("nc", causal)
    if key not in _CACHE:
        _CACHE[key] = _build_nc(causal)
    return _CACHE[key]


def _bf(a):
    return np.ascontiguousarray(a, dtype=np.float32).astype(BF16)


def _wperm(wT, nko):
    """[nko*128, M] -> [128, nko, M] so each SBUF partition's data is one
    contiguous run in DRAM (single DMA descriptor per partition)."""
    wT = np.asarray(wT, np.float32)
    m = wT.shape[1]
    return np.ascontiguousarray(
        wT.reshape(nko, P, m).transpose(1, 0, 2)).astype(BF16)


def kernel(q, k, v, mask, Wq, bq, Wk, bk, Wv, bv, Wo, bo):
    q = np.asarray(q, np.float32)
    k = np.asarray(k, np.float32)
    v = np.asarray(v, np.float32)
    mask = np.asarray(mask)
    Wq, bq = np.asarray(Wq, np.float32), np.asarray(bq, np.float32)
    Wk, bk = np.asarray(Wk, np.float32), np.asarray(bk, np.float32)
    Wv, bv = np.asarray(Wv, np.float32), np.asarray(bv, np.float32)
    Wo, bo = np.asarray(Wo, np.float32), np.asarray(bo, np.float32)

    m2 = mask.reshape(S, S) != 0
    if m2.all():
        causal = False
    else:
        tri = np.tril(np.ones((S, S), bool))
        assert (m2 == tri).all(), "only causal or all-ones masks supported"
        causal = True

    nc = _get_nc(causal)

    cm1 = np.asarray(
        np.arange(P)[:, None] <= np.arange(P)[None, :], np.float32
    ).astype(BF16)  # [k, q] keep-region of the diagonal 128-band
    cm = np.ascontiguousarray(np.stack([cm1, cm1], axis=1))  # [P, 2, P]

    xT = {}
    for b in range(B):
        xT[("q", b)] = _bf(q[b].T)
        xT[("k", b)] = _bf(k[b].T)
        xT[("v", b)] = _bf(v[b].T)

    in_maps = []
    for c in range(NCORES):
        b = c // 4
        rows = slice((c % 4) * DC, (c % 4) * DC + DC)
        bq_s = (bq[rows] / SCALE).reshape(2, P).T
        bk_s = bk[rows].reshape(2, P).T
        in_maps.append({
            "xqT": xT[("q", b)],
            "xkT": xT[("k", b)],
            "xvT": xT[("v", b)],
            "wqT": _wperm(Wq[rows].T / SCALE, 8),
            "wkT": _wperm(Wk[rows].T, 8),
            "wvT": _wperm(Wv[rows].T, 8),
            "woT": _wperm(Wo[:, rows].T, 2),
            "bqk": np.ascontiguousarray(
                np.concatenate([bq_s, bk_s], axis=1), np.float32),
            "cmask": cm,
        })

    res = run_bass_kernel_spmd(nc, in_maps, core_ids=list(range(NCORES)))
    LAST["exec_time_ns"] = res.exec_time_ns
    LAST["results"] = res

    host_bias = (bo + bv @ Wo.T).astype(np.float32)
    out = np.zeros((B, S, E), np.float32)
    for c in range(NCORES):
        out[c // 4] += res.results[c]["out"].astype(np.float32)
    out += host_bias
    return out


# revision 32
# speedup vs baseline: 1.1581x; 1.1581x over previous
"""Multi-head causal self-attention on 8 Trainium2 NeuronCores.

Problem: B=2, S=2048, E=1024, H=16 heads (D=64), causal mask, f32 I/O.

Sharding: (batch x head-group) -> 8 cores. Core c handles batch b=c//4 and
4 heads h0=4*(c%4).. (column-parallel Q/K/V projections, local attention,
row-parallel partial output projection). The 4 partial outputs per batch are
summed on the host (the "all-reduce" of row-parallel TP), where the output
bias bo and the folded V-bias term (bv @ Wo.T, exact because softmax rows
sum to 1) are also added. Partial outputs travel as bf16 (summed in f32 on
the host); the rel-error budget absorbs the rounding.

The attention phase is ScalarE-exp-throughput-bound (~75us of EXP), so the
program is software-pipelined so every other engine's work hides behind the
exp stream:
  - scores for a head pair land in one 2-bank PSUM tile [128, 2, 512];
    ONE ScalarE exp instruction covers both heads (ACT count matters).
  - The AV matmuls are emitted LAG=2 k-blocks behind the QK/exp front, so
    they never stall the PE queue waiting on an exp that hasn't run.
  - The causal-diagonal masking multiplies run on the otherwise-idle GpSimd
    engine (exact: exp(s)*0 == 0).
  - Each q-superblock's output projection + drain is spread into the next
    superblock's kb loop, keeping ScalarE fed during the PE-side work.

Device kernel layout (all matmuls bf16 with f32 PSUM accumulate):
  - Host pre-transposes activations/weights so the kernel never transposes:
      qhT/khT = Wq_h @ q[b].T  (projection emits [d, s] directly)
  - khT is per-head zero-padded: [128, 4, S]; head h occupies partitions
    (h%2)*64..+64 of slot h, the complementary half is zeros. qhT is packed
    as pair-slots [128, 2, S] (head 2m in partitions 0:64, head 2m+1 in
    64:128): the QK matmul contracts khT's zero half against the other
    head's q data, which contributes exactly zero. This keeps the Q-side
    PSUM drain a single full-width tensor_scalar_add.
  - attn^T feeds AV as the moving operand:
      ctx^T [d, q] = matmul(lhsT=V_aug [k, 128], rhs=attn^T)
    where V_aug cols 64:128 are ones, so rows 64:127 of the AV psum are the
    softmax row-sums pre-broadcast across 64 partitions (DVE cannot
    broadcast along partitions). Normalization is a DVE reciprocal+mul.
  - 1/sqrt(D) is folded into Wq/bq on the host.
  - Causal structure is exploited: only lower-triangular k-blocks are
    computed.
"""

import os
import sys

for _p in ("/opt/trn_rl_repo",):
    if _p not in sys.path and os.path.isdir(_p):
        sys.path.insert(0, _p)

import numpy as np
import ml_dtypes

import concourse.bacc as bacc
from concourse import mybir
from concourse.tile import TileContext
from concourse.bass_utils import run_bass_kernel_spmd

BF16 = ml_dtypes.bfloat16
P = 128
B, S, E, H, D = 2, 2048, 1024, 16, 64
HPC = 4            # heads per core
DC = HPC * D       # 256 output dims per core per projection
NCORES = 8
QSUP = 512         # q-superblock (matmul free dim)
NSUP = S // QSUP   # 4
NKB = S // P       # 16 k-blocks
LAG = 2            # AV runs LAG k-blocks behind the QK/exp front
SCALE = float(np.sqrt(D))

AF = mybir.ActivationFunctionType
f32 = mybir.dt.float32
bf16 = mybir.dt.bfloat16

_CACHE = {}
LAST = {}


def _install_axon_profile_shim():
    """Provide antenv.axon_hooks (absent in this image) so
    run_bass_kernel_spmd(trace=True) can NTFF-profile via libaxon_pjrt.so."""
    try:
        import antenv.axon_hooks  # noqa: F401
        return
    except ImportError:
        pass
    import contextlib
    import ctypes
    import types

    import antenv

    state = {"hook": None, "tried": False}

    def _build_hook():
        so_path = "/opt/axon/libaxon_pjrt.so"
        if not os.path.exists(so_path):
            return None
        lib = ctypes.CDLL(so_path)
        if not hasattr(lib, "axon_start_nrt_profile"):
            return None
        lib.axon_start_nrt_profile.argtypes = [
            ctypes.POINTER(ctypes.c_int64),
            ctypes.c_size_t,
        ]
        lib.axon_start_nrt_profile.restype = ctypes.c_int64
        lib.axon_stop_nrt_profile.argtypes = [ctypes.c_char_p]
        lib.axon_stop_nrt_profile.restype = ctypes.c_int64

        @contextlib.contextmanager
        def _hook(output_dir, device_ids):
            import jax

            jax.devices()
            if device_ids:
                ids = (ctypes.c_int64 * len(device_ids))(*device_ids)
                rc = lib.axon_start_nrt_profile(ids, len(device_ids))
            else:
                rc = lib.axon_start_nrt_profile(None, 0)
            if rc != 0:
                raise RuntimeError(f"axon_start_nrt_profile rc={rc}")
            try:
                yield
            finally:
                n = lib.axon_stop_nrt_profile(str(output_dir).encode())
                if n < 0:
                    raise RuntimeError(f"axon_stop_nrt_profile rc={n}")
                print(f"profile: {n} file(s) written to {output_dir}")

        return _hook

    mod = types.ModuleType("antenv.axon_hooks")

    def set_axon_ntff_profile_hook(h):
        state["hook"] = h
        state["tried"] = True

    def get_axon_ntff_profile_hook():
        if not state["tried"]:
            state["hook"] = _build_hook()
            state["tried"] = True
        return state["hook"]

    mod.set_axon_ntff_profile_hook = set_axon_ntff_profile_hook
    mod.get_axon_ntff_profile_hook = get_axon_ntff_profile_hook
    sys.modules["antenv.axon_hooks"] = mod
    antenv.axon_hooks = mod


_install_axon_profile_shim()


def _build_nc(causal: bool):
    nc = bacc.Bacc(None, target_bir_lowering=False)

    xqT = nc.dram_tensor("xqT", [E, S], bf16, kind="ExternalInput")
    xkT = nc.dram_tensor("xkT", [E, S], bf16, kind="ExternalInput")
    xvT = nc.dram_tensor("xvT", [E, S], bf16, kind="ExternalInput")
    wqT = nc.dram_tensor("wqT", [P, 8, DC], bf16, kind="ExternalInput")
    wkT = nc.dram_tensor("wkT", [P, 8, DC], bf16, kind="ExternalInput")
    wvT = nc.dram_tensor("wvT", [P, 8, DC], bf16, kind="ExternalInput")
    woT = nc.dram_tensor("woT", [P, 2, E], bf16, kind="ExternalInput")
    bqk = nc.dram_tensor("bqk", [P, 4], f32, kind="ExternalInput")
    cmask = nc.dram_tensor("cmask", [P, 2, P], bf16, kind="ExternalInput")
    out = nc.dram_tensor("out", [S, E], bf16, kind="ExternalOutput")

    with TileContext(nc) as tc:
        with (
            tc.tile_pool(name="consts", bufs=1) as consts,
            tc.tile_pool(name="xin", bufs=24) as xin,
            tc.tile_pool(name="acts", bufs=1) as acts,
            tc.tile_pool(name="attn", bufs=4) as attn,
            tc.tile_pool(name="norm", bufs=2) as norm,
            tc.tile_pool(name="osb", bufs=3) as osb,
            tc.tile_pool(name="ppool", bufs=2, space="PSUM") as ppool,
            tc.tile_pool(name="stp", bufs=2, space="PSUM") as stp,
            tc.tile_pool(name="cpool", bufs=2, space="PSUM") as cpool,
        ):
            # ---- HAM warm-up -----------------------------------------------
            # A dependency-free burst of matmuls on a zeroed scratch tile
            # warms the PE clock gate (~3.4us of activity needed) while the
            # first input DMAs stream (results are never read).
            warm = consts.tile([P, QSUP], bf16)
            nc.vector.memset(warm[:], 0.0)
            for wi in range(9):
                wp = ppool.tile([P, QSUP], f32, tag="ps", name=f"warm_{wi}")
                nc.tensor.matmul(wp, warm[:, 0:P], warm[:], start=True,
                                 stop=True)

            # ---- constants -------------------------------------------------
            wq_sb = consts.tile([P, 8, DC], bf16)
            wk_sb = consts.tile([P, 8, DC], bf16)
            wv_sb = consts.tile([P, 8, DC], bf16)
            wo_sb = consts.tile([P, 2, E], bf16)
            nc.sync.dma_start(wq_sb, wqT[:])
            bqk_sb = consts.tile([P, 4], f32)
            nc.sync.dma_start(bqk_sb[:], bqk[:])
            if causal:
                cm_sb = consts.tile([P, 2, P], bf16)
                nc.sync.dma_start(cm_sb[:], cmask[:])

            # ---- activations ----------------------------------------------
            # qhT pair-slots: [128, m, S]; head 2m in partitions 0:64, head
            # 2m+1 in 64:128 (matches the bias layout -> one full-width
            # tensor_scalar_add drains each Q PSUM chain).
            qhT = acts.tile([P, 2, S], bf16)
            # khT per-head zero-padded slots (the stationary side must be
            # zero-padded so the QK contraction kills the other head's data
            # in qhT). Pads are zeroed on the idle GpSimd engine; they are
            # only read ~50us later by the attention phase.
            khT = acts.tile([P, 4, S], bf16)
            for h in range(4):
                if h % 2 == 0:
                    nc.gpsimd.memset(khT[D:, h, :], 0.0)
                else:
                    nc.gpsimd.memset(khT[0:D, h, :], 0.0)
            # V natural layout + ones block: [:, sb, h, 0:64] = vh, 64:128 ones
            vha = acts.tile([P, NKB, HPC, 2 * D], bf16)
            ctxT = acts.tile([P, 2, S], bf16)
            nc.vector.memset(vha[:, :, :, D:], 1.0)

            # ---- Q/K/V projections ----------------------------------------
            def load_x(xT, t):
                xr = xT.rearrange("(ko p) s -> ko p s", p=P)
                tiles = []
                for ko in range(8):
                    tile = xin.tile([P, S], bf16, tag="xin",
                                    name=f"x_{t}_{ko}")
                    nc.sync.dma_start(tile, xr[ko])
                    tiles.append(tile)
                return tiles

            xt_q = load_x(xqT, "q")
            xt_k = load_x(xkT, "k")
            nc.sync.dma_start(wk_sb, wkT[:])
            nc.sync.dma_start(wv_sb, wvT[:])
            nc.sync.dma_start(wo_sb, woT[:])
            xt_v = load_x(xvT, "v")

            # ko-outer multi-chain projection pass: `chains[i]` accumulate
            # over ko as the x tiles land from DMA (tile ko is read right
            # when it arrives - the first pass is DMA-paced, not serialized
            # after it). Uses ps+cps PSUM slots; pre-attention only.
            def proj_pass(w_sb, xt, specs, drain):
                # specs: list of (name, x column slice); drain(i, chain)
                chains = []
                for i, (nm, _) in enumerate(specs):
                    pool, tag = ((ppool, "ps"), (cpool, "cps"))[i % 2]
                    chains.append(pool.tile([P, QSUP], f32, tag=tag,
                                            name=nm))
                for ko in range(8):
                    for i, (_, csl) in enumerate(specs):
                        nc.tensor.matmul(
                            chains[i],
                            w_sb[0][:, ko, w_sb[1] * P:(w_sb[1] + 1) * P]
                            if w_sb[1] is not None else w_sb[0][:, ko, :],
                            xt[ko][:, csl],
                            start=(ko == 0), stop=(ko == 7),
                        )
                for i in range(len(specs)):
                    drain(i, chains[i])

            def q_drain(m):
                def dr(i, ch):
                    nsl = slice(i * QSUP, (i + 1) * QSUP)
                    nc.vector.tensor_scalar_add(
                        qhT[:, m, nsl], ch, bqk_sb[:, m:m + 1],
                    )
                return dr

            def k_drain(m, base):
                def dr(i, ch):
                    ns = base + i
                    nsl = slice(ns * QSUP, (ns + 1) * QSUP)
                    bsl = bqk_sb[:, 2 + m:3 + m]
                    nc.vector.tensor_scalar_add(
                        khT[0:D, 2 * m, nsl], ch[0:D], bsl[0:D],
                    )
                    nc.vector.tensor_scalar_add(
                        khT[D:, 2 * m + 1, nsl], ch[D:], bsl[D:],
                    )
                return dr

            def kproj_chunk(ns, m):
                # single-chain filler form (x tiles already resident)
                def go():
                    ch = ppool.tile([P, QSUP], f32, tag="ps",
                                    name=f"k_{ns}_{m}")
                    for ko in range(8):
                        nc.tensor.matmul(
                            ch,
                            wk_sb[:, ko, m * P:(m + 1) * P],
                            xt_k[ko][:, ns * QSUP:(ns + 1) * QSUP],
                            start=(ko == 0), stop=(ko == 7),
                        )
                    k_drain(m, ns)(0, ch)
                return go

            def vproj_chunk(sb, pre=False):
                def go():
                    nm = f"vps_{sb}"
                    if pre:
                        pool, tag = ((ppool, "ps"), (cpool, "cps"))[sb % 2]
                    else:
                        pool, tag = ppool, "ps"
                    ps = pool.tile([P, DC], f32, tag=tag, name=nm)
                    for ko in range(8):
                        nc.tensor.matmul(
                            ps,
                            xt_v[ko][:, sb * P:(sb + 1) * P],
                            wv_sb[:, ko, :],
                            start=(ko == 0), stop=(ko == 7),
                        )
                    nc.vector.tensor_copy(
                        vha[:, sb, :, 0:D],
                        ps.rearrange("p (h d) -> p h d", h=HPC),
                    )
                return go

            def outproj_chunk(sb, split_drain=False, alt_pool=False):
                def go():
                    if alt_pool:
                        # tail-only: st2 banks are free once the last exp
                        # ran; alternating pools lets consecutive chunks'
                        # matmuls run back-to-back instead of waiting on
                        # the previous chunk's drain
                        ps2 = stp.tile([P, 2, QSUP], f32, tag="st2",
                                       name=f"pso2_{sb}")
                        pso = [ps2[:, 0, :], ps2[:, 1, :]]
                    else:
                        pso = [
                            ppool.tile([P, QSUP], f32, tag="ps",
                                       name=f"pso_{sb}_{n2}")
                            for n2 in range(2)
                        ]
                    for km in range(2):
                        for n2 in range(2):
                            nc.tensor.matmul(
                                pso[n2],
                                ctxT[:, km, sb * P:(sb + 1) * P],
                                wo_sb[:, km, n2 * QSUP:(n2 + 1) * QSUP],
                                start=(km == 0), stop=(km == 1),
                            )
                    # drain both halves into one [128, 1024] tile and DMA
                    # full DRAM rows: 2KB descriptors, half the descriptor
                    # count of two 512-wide stores
                    ot = osb.tile([P, 2, QSUP], bf16, tag="ot",
                                  name=f"ot_{sb}")
                    nc.vector.tensor_copy(out=ot[:, 0, :], in_=pso[0])
                    if split_drain:
                        # ScalarE is idle in the tail; halve the
                        # drain-copy serialization
                        nc.scalar.activation(ot[:, 1, :], pso[1], AF.Copy)
                    else:
                        nc.vector.tensor_copy(out=ot[:, 1, :], in_=pso[1])
                    nc.sync.dma_start(
                        out[sb * P:(sb + 1) * P, :],
                        ot.rearrange("p a q -> p (a q)"),
                    )
                return go

            # Up-front (DMA-paced, ko-outer multi-chain): full Q, K ns=0..1.
            # V interleaves with qs=0's QK/exp below; K ns=2..3, V sb=8..15
            # and every output projection are drip-fed into the attention
            # kb loops: pure-PE work that keeps the array busy while the
            # ScalarE exp stream (the attention bottleneck) drains.
            for m in range(2):
                proj_pass((wq_sb, m), xt_q,
                          [(f"q_{ns}_{m}",
                            slice(ns * QSUP, (ns + 1) * QSUP))
                           for ns in range(4)],
                          q_drain(m))
            for m in range(2):
                proj_pass((wk_sb, m), xt_k,
                          [(f"k_{ns}_{m}",
                            slice(ns * QSUP, (ns + 1) * QSUP))
                           for ns in range(2)],
                          k_drain(m, 0))

            def qlo_of(kb, qs):
                r = kb - 4 * qs
                return r * P if (causal and r >= 0) else 0

            def emit_qk(qs, m, kb):
                qlo = qlo_of(kb, qs)
                qsl = slice(qs * QSUP + qlo, (qs + 1) * QSUP)
                st = stp.tile([P, 2, QSUP], f32, tag="st2",
                              name=f"st_{m}_{qs}_{kb}")
                for h2 in range(2):
                    nc.tensor.matmul(
                        st[:, h2, qlo:],
                        khT[:, 2 * m + h2, kb * P:(kb + 1) * P],
                        qhT[:, m, qsl],
                        start=True, stop=True,
                    )
                at = attn.tile([P, 2, QSUP], bf16, tag="at",
                               name=f"at_{m}_{qs}_{kb}")
                nc.scalar.activation(at[:, :, qlo:], st[:, :, qlo:],
                                     AF.Exp)
                if causal and kb - 4 * qs >= 0:
                    nc.gpsimd.tensor_mul(
                        at[:, :, qlo:qlo + P], at[:, :, qlo:qlo + P],
                        cm_sb,
                    )
                return at

            def emit_av(qs, m, kb, cps, at, nkb):
                qlo = qlo_of(kb, qs)
                for h2 in range(2):
                    nc.tensor.matmul(
                        cps[h2][:, qlo:],
                        vha[:, kb, 2 * m + h2, :],
                        at[:, h2, qlo:],
                        start=(kb == 0), stop=(kb == nkb - 1),
                    )

            def mk_cps(qs, m):
                return [
                    cpool.tile([P, QSUP], f32, tag="cps",
                               name=f"cps_{m}_{qs}_{h2}")
                    for h2 in range(2)
                ]

            def emit_norm(qs, m, cps):
                # drain ctx + row-sums to SBUF first (4 half-width copies
                # free the cps accumulators for the next head pair ASAP),
                # then one reciprocal + one full-width mul into ctxT.
                cu = norm.tile([P, QSUP], bf16, tag="cu")
                su = norm.tile([P, QSUP], f32, tag="sums")
                nc.vector.tensor_copy(out=cu[0:D, :], in_=cps[0][0:D, :])
                nc.vector.tensor_copy(out=cu[D:, :], in_=cps[1][0:D, :])
                nc.vector.tensor_copy(out=su[0:D, :], in_=cps[0][D:, :])
                nc.vector.tensor_copy(out=su[D:, :], in_=cps[1][D:, :])
                rec = norm.tile([P, QSUP], f32, tag="rec")
                nc.vector.reciprocal_approx_fast(out=rec, in_=su)
                qsl = slice(qs * QSUP, (qs + 1) * QSUP)
                nc.vector.tensor_mul(ctxT[:, m, qsl], cu, rec)

            # fillers[(qs, m)] -> list of chunks to drip into that kb loop
            fillers = {}
            if causal:
                # K ns=2 fills the PE while the first V chunks wait on the
                # xv DMA (V is DMA-gated there regardless)
                kproj_chunk(2, 0)()
                kproj_chunk(2, 1)()
                for sb in range(8):
                    vproj_chunk(sb, pre=True)()
                fillers[(0, 0)] = [vproj_chunk(8), vproj_chunk(9),
                                   vproj_chunk(10)]
                fillers[(0, 1)] = [vproj_chunk(11), vproj_chunk(12)]
                fillers[(1, 0)] = ([outproj_chunk(sb) for sb in range(4)]
                                   + [kproj_chunk(3, 0), kproj_chunk(3, 1)])
                fillers[(1, 1)] = [vproj_chunk(13), vproj_chunk(14),
                                   vproj_chunk(15)]
                fillers[(2, 0)] = [outproj_chunk(sb) for sb in range(4, 8)]
                fillers[(3, 0)] = [outproj_chunk(sb) for sb in range(8, 12)]
                tail = [outproj_chunk(sb, split_drain=True)
                        for sb in range(12, 16)]
                qs_start = 0
            else:
                for m in range(2):
                    proj_pass((wk_sb, m), xt_k,
                              [(f"k_{ns}_{m}",
                                slice(ns * QSUP, (ns + 1) * QSUP))
                               for ns in range(2, 4)],
                              k_drain(m, 2))
                for sb in range(NKB):
                    vproj_chunk(sb, pre=True)()
                fillers[(1, 0)] = [outproj_chunk(sb) for sb in range(4)]
                fillers[(2, 0)] = [outproj_chunk(sb) for sb in range(4, 8)]
                fillers[(3, 0)] = [outproj_chunk(sb) for sb in range(8, 12)]
                tail = [outproj_chunk(sb, split_drain=True)
                        for sb in range(12, 16)]
                qs_start = 0

            # ---- attention -------------------------------------------------
            for qs in range(qs_start, NSUP):
                for m in range(2):        # head pair (local heads 2m, 2m+1)
                    nkb = 4 * qs + 4 if causal else NKB
                    drip = list(fillers.get((qs, m), []))
                    cps = mk_cps(qs, m)
                    ats = {}
                    for kb in range(nkb + LAG):
                        if kb < nkb:
                            ats[kb] = emit_qk(qs, m, kb)
                        if drip and kb >= 1:
                            drip.pop(0)()
                        j = kb - LAG
                        if 0 <= j < nkb:
                            emit_av(qs, m, j, cps, ats.pop(j), nkb)
                    for chunk in drip:   # leftovers (short kb loop)
                        chunk()
                    emit_norm(qs, m, cps)

            for chunk in tail:
                chunk()

    nc.finalize()
    return nc


def _get_nc(causal: bool):
    key = ("nc", causal)
    if key not in _CACHE:
        _CACHE[key] = _build_nc(causal)
    return _CACHE[key]


def _bf(a):
    return np.ascontiguousarray(a, dtype=np.float32).astype(BF16)


def _wperm(wT, nko):
    """[nko*128, M] -> [128, nko, M] so each SBUF partition's data is one
    contiguous run in DRAM (single DMA descriptor per partition)."""
    wT = np.asarray(wT, np.float32)
    m = wT.shape[1]
    return np.ascontiguousarray(
        wT.reshape(nko, P, m).transpose(1, 0, 2)).astype(BF16)


def kernel(q, k, v, mask, Wq, bq, Wk, bk, Wv, bv, Wo, bo):
    q = np.asarray(q, np.float32)
    k = np.asarray(k, np.float32)
    v = np.asarray(v, np.float32)
    mask = np.asarray(mask)
    Wq, bq = np.asarray(Wq, np.float32), np.asarray(bq, np.float32)
    Wk, bk = np.asarray(Wk, np.float32), np.asarray(bk, np.float32)
    Wv, bv = np.asarray(Wv, np.float32), np.asarray(bv, np.float32)
    Wo, bo = np.asarray(Wo, np.float32), np.asarray(bo, np.float32)

    m2 = mask.reshape(S, S) != 0
    if m2.all():
        causal = False
    else:
        tri = np.tril(np.ones((S, S), bool))
        assert (m2 == tri).all(), "only causal or all-ones masks supported"
        causal = True

    nc = _get_nc(causal)

    cm1 = np.asarray(
        np.arange(P)[:, None] <= np.arange(P)[None, :], np.float32
    ).astype(BF16)  # [k, q] keep-region of the diagonal 128-band
    cm = np.ascontiguousarray(np.stack([cm1, cm1], axis=1))  # [P, 2, P]

    xT = {}
    for b in range(B):
        xT[("q", b)] = _bf(q[b].T)
        xT[("k", b)] = _bf(k[b].T)
        xT[("v", b)] = _bf(v[b].T)

    in_maps = []
    for c in range(NCORES):
        b = c // 4
        rows = slice((c % 4) * DC, (c % 4) * DC + DC)
        bq_s = (bq[rows] / SCALE).reshape(2, P).T
        bk_s = bk[rows].reshape(2, P).T
        in_maps.append({
            "xqT": xT[("q", b)],
            "xkT": xT[("k", b)],
            "xvT": xT[("v", b)],
            "wqT": _wperm(Wq[rows].T / SCALE, 8),
            "wkT": _wperm(Wk[rows].T, 8),
            "wvT": _wperm(Wv[rows].T, 8),
            "woT": _wperm(Wo[:, rows].T, 2),
            "bqk": np.ascontiguousarray(
                np.concatenate([bq_s, bk_s], axis=1), np.float32),
            "cmask": cm,
        })

    res = run_bass_kernel_spmd(nc, in_maps, core_ids=list(range(NCORES)))
    LAST["exec_time_ns"] = res.exec_time_ns
    LAST["results"] = res

    host_bias = (bo + bv @ Wo.T).astype(np.float32)
    out = np.zeros((B, S, E), np.float32)
    for c in range(NCORES):
        out[c // 4] += res.results[c]["out"].astype(np.float32)
    out += host_bias
    return out
